# revision 1
# baseline (speedup 1.0000x reference)
"""Embedding lookup (weight[indices]) on 8 TRN2 NeuronCores.

Strategy: replicate the 1M x 128 f32 table in each core's HBM, shard the
4096*200 = 819200 indices 8 ways (data parallel).  Each core loops over
tiles of 128*K indices: an SWDGE indirect DMA gathers 128*K rows (512 B
each) from the HBM table into an SBUF tile [128, K*128], then an HWDGE
DMA stores the tile to the core's contiguous output shard.  Tile
framework handles double buffering and semaphores.
"""

import numpy as np

NUM_EMB = 1_000_000
D = 128
N_CORES = 8
P = 128

# tuning knobs
K = 50        # rows gathered per partition per tile -> tile = [128, K*128] f32
BUFS = 4      # SBUF tile double-buffering depth

_CACHE = {}


def _build_bass(per_core: int, k: int, bufs: int, num_emb: int = NUM_EMB, reps: int = 1):
    import concourse.bacc as bacc
    import concourse.bass as bass
    import concourse.mybir as mybir
    import concourse.tile as tile

    key = (per_core, k, bufs, num_emb, reps)
    if key in _CACHE:
        return _CACHE[key]

    nc = bacc.Bacc(
        "TRN2",
        target_bir_lowering=False,
        debug=False,
        num_devices=N_CORES,
    )
    idx = nc.dram_tensor("idx", [per_core], mybir.dt.int32, kind="ExternalInput")
    weight = nc.dram_tensor(
        "weight", [num_emb, D], mybir.dt.float32, kind="ExternalInput"
    )
    out = nc.dram_tensor("out", [per_core, D], mybir.dt.float32, kind="ExternalOutput")

    n_per_part = per_core // P            # indices each partition handles
    assert per_core == n_per_part * P
    n_tiles = n_per_part // k
    assert n_per_part == n_tiles * k

    with tile.TileContext(nc) as tc:
        with (
            tc.tile_pool(name="idxp", bufs=1) as idxp,
            tc.tile_pool(name="data", bufs=bufs) as datap,
        ):
            idx_tile = idxp.tile([P, n_per_part], mybir.dt.int32)
            nc.sync.dma_start(idx_tile[:], idx[:].rearrange("(p n) -> p n", p=P))
            out_r = out[:].rearrange("(p n) d -> p (n d)", p=P)

            def body():
                for t in range(n_tiles):
                    dtile = datap.tile([P, k * D], mybir.dt.float32)
                    # HW only supports the [128,1]-offset + per-partition-D-run
                    # indirect pattern: one offset per partition, D contiguous
                    # elements each.  So k gathers of 128 rows fill the tile.
                    for j in range(k):
                        n = t * k + j
                        nc.gpsimd.indirect_dma_start(
                            out=dtile[:, j * D : (j + 1) * D],
                            out_offset=None,
                            in_=weight[:],
                            in_offset=bass.IndirectOffsetOnAxis(
                                ap=idx_tile[:, n : n + 1], axis=0
                            ),
                        )
                    nc.sync.dma_start(
                        out_r[:, t * k * D : (t + 1) * k * D], dtile[:]
                    )

            if reps == 1:
                body()
            else:
                with tc.For_i(0, reps, 1):
                    body()
    nc.compile()
    _CACHE[key] = nc
    return nc


def run_sharded(indices: np.ndarray, weight: np.ndarray, trace: bool = False):
    """Shard indices across 8 cores, run the Bass kernel, return
    (full_output, BassKernelResults)."""
    from concourse.bass_utils import run_bass_kernel_spmd

    idx_flat = np.ascontiguousarray(indices.reshape(-1).astype(np.int32))
    w = np.ascontiguousarray(weight, dtype=np.float32)
    n_idx = idx_flat.shape[0]
    per_core = n_idx // N_CORES
    assert n_idx == per_core * N_CORES

    nc = _build_bass(per_core, K, BUFS)
    in_maps = [
        {"idx": idx_flat[c * per_core : (c + 1) * per_core], "weight": w}
        for c in range(N_CORES)
    ]
    res = run_bass_kernel_spmd(
        nc, in_maps, core_ids=list(range(N_CORES)), trace=trace
    )
    # per-core output rows are ordered [p * n_per_part + n] -> global order
    # within the shard matches the input order (we sharded contiguously).
    full = np.concatenate([r["out"] for r in res.results], axis=0)
    return full.reshape(indices.shape + (D,)), res


def kernel(indices: np.ndarray, weight: np.ndarray) -> np.ndarray:
    full, _ = run_sharded(indices, weight, trace=False)
    return full

